# revision 1
# baseline (speedup 1.0000x reference)
"""Trainium2 Bass kernel for the RNN-T style Joint network:

    out[b,t,u,v] = sum_k tanh(enc_p[b,t,k] + dec_p[b,u,k] + b1[k]) * W2[v,k] + b2[v]
    enc_p = h_enc @ W1[:, :H].T ; dec_p = h_dec @ W1[:, H:].T

Sharding: data-parallel over B across 8 NeuronCores (B == 8, one batch row per
core). Weights are replicated. No collectives needed.

Per-core pipeline (one NeuronCore):
  GEMM1 (fp32, PE): enc_pT [HID, T] and dec_pT [HID, U] computed directly in
      transposed layout (HID on partitions); b1 folded in via the ScalarE
      per-partition activation bias during PSUM->SBUF evacuation.
  broadcast-add (VectorE): pre[j, t'*64+u] = encbT[j, t] + decT[j, u] in ONE
      tensor_add per [128, 512] block using stride-0 broadcast access
      patterns (verified supported by the DVE).
  tanh (ScalarE): SBUF fp32 -> SBUF bf16, producing hT [HID, TU-chunk] --
      already transposed to be the stationary operand of GEMM2.
  GEMM2 (PE, bf16): out[tu, v] = hT.T @ W2T accumulated over 5 K-tiles in
      fp32 PSUM (1280 N=512 matmuls: the roofline term).
  b2 add (VectorE): PSUM + b2rep -> SBUF fp32 out tile [128, 1024].
  DMA out: contiguous 512KB stores.

The build for chunk c+2 is emitted before GEMM2 of chunk c so the in-order
VectorE queue always runs the next chunk's broadcast-adds ahead of the
current chunk's evacuations, keeping the PE from stalling on hT tiles.
"""

import numpy as np
import ml_dtypes

B, T, U, H = 8, 256, 64, 512
HID, V = 640, 1024
TU = T * U  # 16384
N_CORES = 8
N_CHUNKS = TU // 1024  # 16 chunks of 16 t-values x 64 u-values
KK = HID // 128  # 5 K-tiles

BF16 = ml_dtypes.bfloat16

_CACHE = {}


def _build_bass():
    import concourse.bass as bass
    import concourse.tile as tile
    from concourse import bacc, mybir

    f32 = mybir.dt.float32
    bf16 = mybir.dt.bfloat16
    Tanh = mybir.ActivationFunctionType.Tanh

    nc = bacc.Bacc("TRN2", target_bir_lowering=False, debug=False,
                   num_devices=N_CORES)

    hencT = nc.dram_tensor("hencT", [H, T], bf16, kind="ExternalInput").ap()
    hdecT = nc.dram_tensor("hdecT", [H, U], bf16, kind="ExternalInput").ap()
    w1T = nc.dram_tensor("w1T", [2 * H, HID], bf16, kind="ExternalInput").ap()
    w2T = nc.dram_tensor("w2T", [HID, V], bf16, kind="ExternalInput").ap()
    b1col = nc.dram_tensor("b1col", [HID, 1], f32, kind="ExternalInput").ap()
    b2rep = nc.dram_tensor("b2rep", [128, V], f32, kind="ExternalInput").ap()
    out = nc.dram_tensor("out", [TU, V], f32, kind="ExternalOutput").ap()

    def bcast3(ap2d, mid):
        """[P, N] AP -> [P, mid, N] with a stride-0 middle dim."""
        return bass.AP(tensor=ap2d.tensor, offset=ap2d.offset,
                       ap=[ap2d.ap[0], [0, mid], ap2d.ap[1]])

    def repeat3(ap2d, inner):
        """[P, N] AP -> [P, N, inner] with a stride-0 inner dim."""
        return bass.AP(tensor=ap2d.tensor, offset=ap2d.offset,
                       ap=[ap2d.ap[0], ap2d.ap[1], [0, inner]])

    with tile.TileContext(nc) as tc:
        with (
            tc.tile_pool(name="consts", bufs=1) as consts,
            tc.tile_pool(name="psum", bufs=1, space="PSUM") as psum,
            tc.tile_pool(name="prep", bufs=4) as prep,
            tc.tile_pool(name="hTp", bufs=3) as hTp,
            tc.tile_pool(name="outp", bufs=4) as outp,
        ):
            # ---- load inputs into SBUF ----
            henc_t = []
            for k in range(4):
                t_ = consts.tile([128, T], bf16, tag=f"hencT{k}", name=f"hencT{k}")
                nc.sync.dma_start(out=t_, in_=hencT[k * 128:(k + 1) * 128, :])
                henc_t.append(t_)
            hdec_t = []
            for k in range(4):
                t_ = consts.tile([128, U], bf16, tag=f"hdecT{k}", name=f"hdecT{k}")
                nc.sync.dma_start(out=t_, in_=hdecT[k * 128:(k + 1) * 128, :])
                hdec_t.append(t_)
            b1_t = []
            for kk in range(KK):
                t_ = consts.tile([128, 1], f32, tag=f"b1{kk}", name=f"b1{kk}")
                nc.sync.dma_start(out=t_, in_=b1col[kk * 128:(kk + 1) * 128, :])
                b1_t.append(t_)
            w1_t = []
            for k in range(8):
                t_ = consts.tile([128, HID], bf16, tag=f"w1T{k}", name=f"w1T{k}")
                nc.sync.dma_start(out=t_, in_=w1T[k * 128:(k + 1) * 128, :])
                w1_t.append(t_)
            w2_t = []
            for k in range(KK):
                t_ = consts.tile([128, V], bf16, tag=f"w2T{k}", name=f"w2T{k}")
                nc.gpsimd.dma_start(out=t_, in_=w2T[k * 128:(k + 1) * 128, :])
                w2_t.append(t_)
            b2_t = consts.tile([128, V], f32, tag="b2", name="b2")
            nc.gpsimd.dma_start(out=b2_t, in_=b2rep[:, :])

            # ---- GEMM1 (fp32): enc_pT [HID, T], dec_pT [HID, U] ----
            encbT = []
            decT = []
            for kk in range(KK):
                ps = psum.tile([128, T], f32, tag="g1", bufs=1, name=f"pse{kk}")
                for k in range(4):
                    nc.tensor.matmul(
                        ps,
                        lhsT=w1_t[k][:, kk * 128:(kk + 1) * 128],
                        rhs=henc_t[k],
                        start=(k == 0), stop=(k == 3),
                    )
                e_ = consts.tile([128, T], f32, tag=f"encbT{kk}", name=f"encbT{kk}")
                # encbT = enc_pT + b1 (per-partition bias)
                nc.scalar.add(out=e_, in_=ps, add=b1_t[kk])
                encbT.append(e_)
                psd = psum.tile([128, U], f32, tag="g1d", bufs=1, name=f"psd{kk}")
                for k in range(4):
                    nc.tensor.matmul(
                        psd,
                        lhsT=w1_t[4 + k][:, kk * 128:(kk + 1) * 128],
                        rhs=hdec_t[k],
                        start=(k == 0), stop=(k == 3),
                    )
                d_ = consts.tile([128, U], f32, tag=f"decT{kk}", name=f"decT{kk}")
                nc.scalar.copy(out=d_, in_=psd)
                decT.append(d_)

            # ---- main loop: build is emitted 2 chunks ahead of GEMM2 ----
            hT_by_chunk = {}

            def emit_build(c):
                hts = []
                for kk in range(KK):
                    pre = prep.tile([128, 1024], f32, tag=f"pre{kk}",
                                    name=f"pre{c}_{kk}", bufs=2)
                    pre_ap = pre[:, :]
                    out3 = bass.AP(tensor=pre_ap.tensor, offset=pre_ap.offset,
                                   ap=[pre_ap.ap[0], [64, 16], [1, 64]])
                    nc.vector.tensor_add(
                        out=out3,
                        in0=bcast3(decT[kk][:, :], 16),
                        in1=repeat3(encbT[kk][:, c * 16:(c + 1) * 16], 64),
                    )
                    ht = hTp.tile([128, 1024], bf16, tag=f"hT{kk}",
                                  name=f"hT{c}_{kk}", bufs=3)
                    nc.scalar.activation(out=ht, in_=pre, func=Tanh)
                    hts.append(ht)
                hT_by_chunk[c] = hts

            emit_build(0)
            emit_build(1)
            for c in range(N_CHUNKS):
                if c + 2 < N_CHUNKS:
                    emit_build(c + 2)
                hts = hT_by_chunk.pop(c)
                for mt in range(8):
                    ot = outp.tile([128, V], f32, tag="out", name=f"out{c}_{mt}")
                    ps2 = psum.tile([128, 1024], f32, tag="g2", bufs=3,
                                    name=f"ps2_{c}_{mt}")
                    for vc in range(2):
                        for kk in range(KK):
                            nc.tensor.matmul(
                                ps2[:, vc * 512:(vc + 1) * 512],
                                lhsT=hts[kk][:, mt * 128:(mt + 1) * 128],
                                rhs=w2_t[kk][:, vc * 512:(vc + 1) * 512],
                                start=(kk == 0), stop=(kk == KK - 1),
                            )
                    nc.vector.tensor_add(out=ot, in0=ps2, in1=b2_t)
                    r0 = c * 1024 + mt * 128
                    nc.sync.dma_start(out=out[r0:r0 + 128, :], in_=ot)

    nc.finalize()
    return nc


def _get_nc():
    if "nc" not in _CACHE:
        _CACHE["nc"] = _build_bass()
    return _CACHE["nc"]


def _make_in_maps(h_enc, h_dec, W1, b1, W2, b2):
    h_enc = np.asarray(h_enc, dtype=np.float32)
    h_dec = np.asarray(h_dec, dtype=np.float32)
    W1 = np.asarray(W1, dtype=np.float32)
    b1 = np.asarray(b1, dtype=np.float32)
    W2 = np.asarray(W2, dtype=np.float32)
    b2 = np.asarray(b2, dtype=np.float32)

    w1T = np.ascontiguousarray(W1.T)                    # [2H, HID] f32
    w2T = np.ascontiguousarray(W2.T).astype(BF16)       # [HID, V] bf16
    b1col = np.ascontiguousarray(b1.reshape(HID, 1))
    b2rep = np.ascontiguousarray(np.tile(b2.reshape(1, V), (128, 1)))

    in_maps = []
    for b in range(N_CORES):
        in_maps.append({
            "hencT": np.ascontiguousarray(h_enc[b].T).astype(BF16),  # [H, T]
            "hdecT": np.ascontiguousarray(h_dec[b].T).astype(BF16),  # [H, U]
            "w1T": w1T.astype(BF16),
            "w2T": w2T,
            "b1col": b1col,
            "b2rep": b2rep,
        })
    return in_maps


def _run(in_maps, **kwargs):
    from concourse import bass_utils
    nc = _get_nc()
    return bass_utils.run_bass_kernel_spmd(
        nc, in_maps, core_ids=list(range(N_CORES)), **kwargs)


def kernel(h_enc, h_dec, W1, b1, W2, b2):
    in_maps = _make_in_maps(h_enc, h_dec, W1, b1, W2, b2)
    res = _run(in_maps)
    outs = [r["out"].reshape(T, U, V) for r in res.results]
    return np.stack(outs, axis=0)



# revision 3
# speedup vs baseline: 1.0146x; 1.0146x over previous
"""Trainium2 Bass kernel for the RNN-T style Joint network:

    out[b,t,u,v] = sum_k tanh(enc_p[b,t,k] + dec_p[b,u,k] + b1[k]) * W2[v,k] + b2[v]
    enc_p = h_enc @ W1[:, :H].T ; dec_p = h_dec @ W1[:, H:].T

Sharding: data-parallel over B across 8 NeuronCores (B == 8, one batch row per
core). Weights are replicated. No collectives needed.

Per-core pipeline (one NeuronCore):
  warmup (PE): 16 dummy N=512 matmuls issued during the input-DMA wait so the
      HAM clock gate reaches K=8/8 before GEMM1.
  input DMA: each tensor packed host-side into a single [128, X] block so one
      dma_start covers it (DMA issue costs ~0.6us each on the issuing queue);
      spread across the sync/vector/gpsimd/scalar queues.
  GEMM1 (fp32->bf16, PE): enc_pT [HID, T] and dec_pT [HID, U] in transposed
      layout, 2 rotating PSUM banks, b1 folded via ScalarE bias during
      evacuation to bf16.
  broadcast-add (VectorE, bf16): pre[j, t'*64+u] = encbT[j, t] + decT[j, u] in
      one tensor_add per [128, 1024] chunk using stride-0 broadcast APs.
  tanh (ScalarE): bf16 -> bf16 hT tiles (stationary operand of GEMM2).
  GEMM2 (PE, bf16): out[tu, v] = hT.T @ W2T, 5 K-tile accumulation in fp32
      PSUM; 1280 N=512 matmuls at ~216ns each are the roofline term.
  b2 add (VectorE): PSUM + b2 -> bf16 out tile [128, 2048] (two mt blocks).
  DMA out: one 512KB store per two mt blocks, alternating sync/gpsimd queues.

Output is written bf16 and upcast to fp32 on the host (adds ~0.1% rms noise;
well inside the 2e-2 gate) to halve the HBM write traffic and SBUF footprint.
"""

import numpy as np
import ml_dtypes

B, T, U, H = 8, 256, 64, 512
HID, V = 640, 1024
TU = T * U  # 16384
N_CORES = 8
N_CHUNKS = TU // 1024  # 16 chunks of 16 t-values x 64 u-values
KK = HID // 128  # 5 K-tiles

BF16 = ml_dtypes.bfloat16

_CACHE = {}


def _build_bass():
    import concourse.bass as bass
    import concourse.tile as tile
    from concourse import bacc, mybir

    f32 = mybir.dt.float32
    bf16 = mybir.dt.bfloat16
    Tanh = mybir.ActivationFunctionType.Tanh

    nc = bacc.Bacc("TRN2", target_bir_lowering=False, debug=False,
                   num_devices=N_CORES)

    hencP = nc.dram_tensor("hencP", [128, 4 * T], bf16, kind="ExternalInput").ap()
    hdecP = nc.dram_tensor("hdecP", [128, 4 * U], bf16, kind="ExternalInput").ap()
    w1P = nc.dram_tensor("w1P", [128, 8 * HID], bf16, kind="ExternalInput").ap()
    w2P = nc.dram_tensor("w2P", [128, KK * V], bf16, kind="ExternalInput").ap()
    b1P = nc.dram_tensor("b1P", [128, KK], f32, kind="ExternalInput").ap()
    b2P = nc.dram_tensor("b2P", [128, V], bf16, kind="ExternalInput").ap()
    out = nc.dram_tensor("out", [TU, V], bf16, kind="ExternalOutput").ap()

    def bcast3(ap2d, mid):
        """[P, N] AP -> [P, mid, N] with a stride-0 middle dim."""
        return bass.AP(tensor=ap2d.tensor, offset=ap2d.offset,
                       ap=[ap2d.ap[0], [0, mid], ap2d.ap[1]])

    def repeat3(ap2d, inner):
        """[P, N] AP -> [P, N, inner] with a stride-0 inner dim."""
        return bass.AP(tensor=ap2d.tensor, offset=ap2d.offset,
                       ap=[ap2d.ap[0], ap2d.ap[1], [0, inner]])

    with tile.TileContext(nc) as tc:
        with (
            tc.tile_pool(name="consts", bufs=1) as consts,
            tc.tile_pool(name="psum", bufs=1, space="PSUM") as psum,
            tc.tile_pool(name="prep", bufs=2) as prep,
            tc.tile_pool(name="hTp", bufs=3) as hTp,
            tc.tile_pool(name="outp", bufs=4) as outp,
        ):
            # ---- PE warmup during the input-DMA wait (HAM -> K=8/8) ----
            wk = consts.tile([128, 512], bf16, tag="wk", name="wk")
            nc.gpsimd.memset(wk, 0)
            for i in range(16):
                pw = psum.tile([128, 512], f32, tag="g1", bufs=2,
                               name=f"warm{i}")
                nc.tensor.matmul(pw, lhsT=wk[:, :128], rhs=wk,
                                 start=True, stop=True)

            # ---- input DMAs: one per tensor, spread over queues ----
            henc_all = consts.tile([128, 4 * T], bf16, tag="henc", name="henc")
            hdec_all = consts.tile([128, 4 * U], bf16, tag="hdec", name="hdec")
            b1_all = consts.tile([128, KK], f32, tag="b1", name="b1")
            w1_all = consts.tile([128, 8 * HID], bf16, tag="w1", name="w1")
            w2_all = consts.tile([128, KK * V], bf16, tag="w2", name="w2")
            b2_t = consts.tile([128, V], bf16, tag="b2", name="b2")
            # enc half of W1 + henc arrive first (GEMM1 critical path)
            nc.sync.dma_start(out=w1_all[:, :4 * HID], in_=w1P[:, :4 * HID])
            nc.scalar.dma_start(out=w1_all[:, 4 * HID:], in_=w1P[:, 4 * HID:])
            nc.gpsimd.dma_start(out=henc_all, in_=hencP[:, :])
            nc.gpsimd.dma_start(out=hdec_all, in_=hdecP[:, :])
            nc.gpsimd.dma_start(out=b1_all, in_=b1P[:, :])
            nc.scalar.dma_start(out=w2_all, in_=w2P[:, :])
            nc.scalar.dma_start(out=b2_t, in_=b2P[:, :])

            # ---- GEMM1: enc_pT [HID, T], dec_pT [HID, U], bf16 evac ----
            encbT = []
            decT = []
            for kk in range(KK):
                pe_ = psum.tile([128, 512], f32, tag="g1", bufs=2,
                                name=f"g1e{kk}")
                for k in range(4):
                    nc.tensor.matmul(
                        pe_[:, :T],
                        lhsT=w1_all[:, k * HID + kk * 128:
                                    k * HID + (kk + 1) * 128],
                        rhs=henc_all[:, k * T:(k + 1) * T],
                        start=(k == 0), stop=(k == 3),
                    )
                e_ = consts.tile([128, T], bf16, tag=f"encbT{kk}",
                                 name=f"encbT{kk}")
                nc.scalar.add(out=e_, in_=pe_[:, :T], add=b1_all[:, kk:kk + 1])
                encbT.append(e_)
                pd_ = psum.tile([128, 512], f32, tag="g1", bufs=2,
                                name=f"g1d{kk}")
                for k in range(4):
                    nc.tensor.matmul(
                        pd_[:, :U],
                        lhsT=w1_all[:, (4 + k) * HID + kk * 128:
                                    (4 + k) * HID + (kk + 1) * 128],
                        rhs=hdec_all[:, k * U:(k + 1) * U],
                        start=(k == 0), stop=(k == 3),
                    )
                d_ = consts.tile([128, U], bf16, tag=f"decT{kk}",
                                 name=f"decT{kk}")
                nc.scalar.copy(out=d_, in_=pd_[:, :U])
                decT.append(d_)

            # ---- main loop: build emitted 2 chunks ahead of GEMM2 ----
            hT_by_chunk = {}

            def emit_build(c):
                hts = []
                for kk in range(KK):
                    pre = prep.tile([128, 1024], bf16, tag=f"pre{kk}",
                                    name=f"pre{c}_{kk}", bufs=2)
                    pre_ap = pre[:, :]
                    out3 = bass.AP(tensor=pre_ap.tensor, offset=pre_ap.offset,
                                   ap=[pre_ap.ap[0], [64, 16], [1, 64]])
                    nc.vector.tensor_add(
                        out=out3,
                        in0=bcast3(decT[kk][:, :], 16),
                        in1=repeat3(encbT[kk][:, c * 16:(c + 1) * 16], 64),
                    )
                    ht = hTp.tile([128, 1024], bf16, tag=f"hT{kk}",
                                  name=f"hT{c}_{kk}", bufs=3)
                    nc.scalar.activation(out=ht, in_=pre, func=Tanh)
                    hts.append(ht)
                hT_by_chunk[c] = hts

            emit_build(0)
            emit_build(1)
            for c in range(N_CHUNKS):
                if c + 2 < N_CHUNKS:
                    emit_build(c + 2)
                hts = hT_by_chunk.pop(c)
                for pair in range(4):
                    ot = outp.tile([128, 2 * V], bf16, tag="out",
                                   name=f"out{c}_{pair}")
                    for half in range(2):
                        mt = pair * 2 + half
                        ps2 = psum.tile([128, 1024], f32, tag="g2", bufs=3,
                                        name=f"ps2_{c}_{mt}")
                        for vc in range(2):
                            for kk in range(KK):
                                nc.tensor.matmul(
                                    ps2[:, vc * 512:(vc + 1) * 512],
                                    lhsT=hts[kk][:, mt * 128:(mt + 1) * 128],
                                    rhs=w2_all[:, kk * V + vc * 512:
                                               kk * V + (vc + 1) * 512],
                                    start=(kk == 0), stop=(kk == KK - 1),
                                )
                        nc.vector.tensor_add(out=ot[:, half * V:(half + 1) * V],
                                             in0=ps2, in1=b2_t)
                    # one DMA per two mt blocks: rows r0..r0+255 of out
                    r0 = c * 1024 + pair * 256
                    ot_ap = ot[:, :]
                    in3 = bass.AP(tensor=ot_ap.tensor, offset=ot_ap.offset,
                                  ap=[ot_ap.ap[0], [V, 2], [1, V]])
                    o3 = bass.AP(tensor=out.tensor, offset=r0 * V,
                                 ap=[[V, 128], [128 * V, 2], [1, V]])
                    q = nc.sync if pair % 2 == 0 else nc.gpsimd
                    q.dma_start(out=o3, in_=in3)

    nc.finalize()
    return nc


def _get_nc():
    if "nc" not in _CACHE:
        _CACHE["nc"] = _build_bass()
    return _CACHE["nc"]


def _make_in_maps(h_enc, h_dec, W1, b1, W2, b2):
    h_enc = np.asarray(h_enc, dtype=np.float32)
    h_dec = np.asarray(h_dec, dtype=np.float32)
    W1 = np.asarray(W1, dtype=np.float32)
    b1 = np.asarray(b1, dtype=np.float32)
    W2 = np.asarray(W2, dtype=np.float32)
    b2 = np.asarray(b2, dtype=np.float32)

    # [2H, HID] -> 8 row-blocks of 128 packed as [128, 8*HID]
    w1T = np.ascontiguousarray(W1.T)
    w1P = np.concatenate([w1T[k * 128:(k + 1) * 128, :] for k in range(8)],
                         axis=1).astype(BF16)
    # [HID, V] -> 5 row-blocks of 128 packed as [128, 5*V]
    w2T = np.ascontiguousarray(W2.T)
    w2P = np.concatenate([w2T[kk * 128:(kk + 1) * 128, :] for kk in range(KK)],
                         axis=1).astype(BF16)
    b1P = np.ascontiguousarray(b1.reshape(KK, 128).T)  # [128, KK] f32
    b2P = np.ascontiguousarray(
        np.tile(b2.reshape(1, V), (128, 1))).astype(BF16)

    in_maps = []
    for b in range(N_CORES):
        hencT = np.ascontiguousarray(h_enc[b].T)  # [H, T]
        hencP = hencT.reshape(4, 128, T).transpose(1, 0, 2).reshape(
            128, 4 * T).astype(BF16)
        hdecT = np.ascontiguousarray(h_dec[b].T)  # [H, U]
        hdecP = hdecT.reshape(4, 128, U).transpose(1, 0, 2).reshape(
            128, 4 * U).astype(BF16)
        in_maps.append({
            "hencP": np.ascontiguousarray(hencP),
            "hdecP": np.ascontiguousarray(hdecP),
            "w1P": w1P,
            "w2P": w2P,
            "b1P": b1P,
            "b2P": b2P,
        })
    return in_maps


def _run(in_maps, **kwargs):
    from concourse import bass_utils
    nc = _get_nc()
    return bass_utils.run_bass_kernel_spmd(
        nc, in_maps, core_ids=list(range(N_CORES)), **kwargs)


def kernel(h_enc, h_dec, W1, b1, W2, b2):
    in_maps = _make_in_maps(h_enc, h_dec, W1, b1, W2, b2)
    res = _run(in_maps)
    outs = [r["out"].reshape(T, U, V).astype(np.float32)
            for r in res.results]
    return np.stack(outs, axis=0)


# revision 4
# speedup vs baseline: 1.0163x; 1.0017x over previous
"""Trainium2 Bass kernel for the RNN-T style Joint network:

    out[b,t,u,v] = sum_k tanh(enc_p[b,t,k] + dec_p[b,u,k] + b1[k]) * W2[v,k] + b2[v]
    enc_p = h_enc @ W1[:, :H].T ; dec_p = h_dec @ W1[:, H:].T

Sharding: data-parallel over B across 8 NeuronCores (B == 8, one batch row per
core). Weights are replicated. No collectives needed.

Per-core pipeline (one NeuronCore):
  warmup (PE): dummy N=512 matmuls during the input-DMA wait keep the HAM
      clock gate at K=8/8 from the start.
  input DMA: W1 packed host-side by GEMM1 k-tile (kk) so each kk's weights
      arrive in their own DMA; chunks round-robin over the sync/scalar/gpsimd
      queues (per-queue transfers serialize at ~45GB/s).
  GEMM1 (PE): per kk as its weights land: enc_pT [HID, T] and dec_pT [HID, U]
      on 2 rotating PSUM banks, b1 folded via ScalarE bias during bf16 evac.
      Build adds for chunks 0/1 interleave per-kk so GEMM2 starts early.
  broadcast-add (VectorE, bf16): pre[j, t'*64+u] = encbT[j, t] + decT[j, u],
      one tensor_add per [128, 1024] chunk via stride-0 broadcast APs.
  tanh (ScalarE): bf16 -> bf16 hT tiles (stationary operand of GEMM2).
  GEMM2 (PE, bf16): 1280 N=512 matmuls at ~216ns each (the roofline term),
      5 K-tile accumulation in fp32 PSUM.
  b2 + evac (VectorE): PSUM + b2 -> bf16 out tile [128, 2048]. For the final
      chunk's odd tiles, b2 is pre-written into PSUM by ScalarE (the matmul
      group then accumulates onto it via has_written) and ScalarE evacuates,
      so the drain after the last matmul runs on both engines in parallel.
  DMA out: one 512KB store per two mt blocks, alternating sync/gpsimd queues.

Output is written bf16 and upcast to fp32 on the host (adds ~0.1% rms noise;
well inside the 2e-2 gate) to halve the HBM write traffic and SBUF footprint.
"""

import numpy as np
import ml_dtypes

B, T, U, H = 8, 256, 64, 512
HID, V = 640, 1024
TU = T * U  # 16384
N_CORES = 8
N_CHUNKS = TU // 1024  # 16 chunks of 16 t-values x 64 u-values
KK = HID // 128  # 5 K-tiles

BF16 = ml_dtypes.bfloat16

_CACHE = {}


def _build_bass():
    import concourse.bass as bass
    import concourse.tile as tile
    from concourse import bacc, mybir

    f32 = mybir.dt.float32
    bf16 = mybir.dt.bfloat16
    Tanh = mybir.ActivationFunctionType.Tanh

    nc = bacc.Bacc("TRN2", target_bir_lowering=False, debug=False,
                   num_devices=N_CORES)

    # W1 enc/dec halves packed as [128, KK, 4, 128] -> [128, KK*512]
    w1eP = nc.dram_tensor("w1eP", [128, KK * 512], bf16, kind="ExternalInput").ap()
    w1dP = nc.dram_tensor("w1dP", [128, KK * 512], bf16, kind="ExternalInput").ap()
    hencP = nc.dram_tensor("hencP", [128, 4 * T], bf16, kind="ExternalInput").ap()
    hdecP = nc.dram_tensor("hdecP", [128, 4 * U], bf16, kind="ExternalInput").ap()
    w2P = nc.dram_tensor("w2P", [128, KK * V], bf16, kind="ExternalInput").ap()
    b1P = nc.dram_tensor("b1P", [128, KK], f32, kind="ExternalInput").ap()
    b2P = nc.dram_tensor("b2P", [128, V], bf16, kind="ExternalInput").ap()
    out = nc.dram_tensor("out", [TU, V], bf16, kind="ExternalOutput").ap()

    def bcast3(ap2d, mid):
        """[P, N] AP -> [P, mid, N] with a stride-0 middle dim."""
        return bass.AP(tensor=ap2d.tensor, offset=ap2d.offset,
                       ap=[ap2d.ap[0], [0, mid], ap2d.ap[1]])

    def repeat3(ap2d, inner):
        """[P, N] AP -> [P, N, inner] with a stride-0 inner dim."""
        return bass.AP(tensor=ap2d.tensor, offset=ap2d.offset,
                       ap=[ap2d.ap[0], ap2d.ap[1], [0, inner]])

    with tile.TileContext(nc) as tc:
        with (
            tc.tile_pool(name="consts", bufs=1) as consts,
            tc.tile_pool(name="psum", bufs=1, space="PSUM") as psum,
            tc.tile_pool(name="prep", bufs=2) as prep,
            tc.tile_pool(name="hTp", bufs=3) as hTp,
            tc.tile_pool(name="outp", bufs=4) as outp,
        ):
            # ---- scratch + PE warmup during the input-DMA wait ----
            wk = consts.tile([128, 512], bf16, tag="wk", name="wk")
            nc.vector.memset(wk, 0)

            def warm_mms(n, label):
                for i in range(n):
                    pw = psum.tile([128, 512], f32, tag="g1", bufs=2,
                                   name=f"warm_{label}_{i}")
                    nc.tensor.matmul(pw, lhsT=wk[:, :128], rhs=wk,
                                     start=True, stop=True)

            warm_mms(10, "pre")

            # ---- input DMAs: chunks round-robin over queues ----
            henc_all = consts.tile([128, 4 * T], bf16, tag="henc", name="henc")
            hdec_all = consts.tile([128, 4 * U], bf16, tag="hdec", name="hdec")
            b1_all = consts.tile([128, KK], f32, tag="b1", name="b1")
            w1e_all = consts.tile([128, KK * 512], bf16, tag="w1e", name="w1e")
            w1d_all = consts.tile([128, KK * 512], bf16, tag="w1d", name="w1d")
            w2_all = consts.tile([128, KK * V], bf16, tag="w2", name="w2")
            b2_t = consts.tile([128, V], bf16, tag="b2", name="b2")

            nc.gpsimd.dma_start(out=b1_all, in_=b1P[:, :])
            # w1 enc kk-chunks alternate sync/scalar; henc right behind
            for kk in range(KK):
                q = nc.sync if kk % 2 == 0 else nc.scalar
                q.dma_start(out=w1e_all[:, kk * 512:(kk + 1) * 512],
                            in_=w1eP[:, kk * 512:(kk + 1) * 512])
            nc.scalar.dma_start(out=henc_all[:, :2 * T], in_=hencP[:, :2 * T])
            nc.sync.dma_start(out=henc_all[:, 2 * T:], in_=hencP[:, 2 * T:])
            nc.gpsimd.dma_start(out=hdec_all, in_=hdecP[:, :])
            # dec weights: small per-kk chunks on gpsimd
            for kk in range(KK):
                nc.gpsimd.dma_start(out=w1d_all[:, kk * 512:(kk + 1) * 512],
                                    in_=w1dP[:, kk * 512:(kk + 1) * 512])
            # w2 in 4 chunks + b2, needed from ~12us
            for j in range(4):
                q = (nc.sync, nc.scalar, nc.gpsimd, nc.sync)[j]
                q.dma_start(out=w2_all[:, j * 1280:(j + 1) * 1280],
                            in_=w2P[:, j * 1280:(j + 1) * 1280])
            nc.scalar.dma_start(out=b2_t, in_=b2P[:, :])

            # ---- GEMM1 per kk (as weights land) + early builds ----
            encbT = []
            decT = []
            hts0 = []
            hts1 = []

            def build_one(c, kk, hts):
                pre = prep.tile([128, 1024], bf16, tag=f"pre{kk}",
                                name=f"pre{c}_{kk}", bufs=2)
                pre_ap = pre[:, :]
                out3 = bass.AP(tensor=pre_ap.tensor, offset=pre_ap.offset,
                               ap=[pre_ap.ap[0], [64, 16], [1, 64]])
                nc.vector.tensor_add(
                    out=out3,
                    in0=bcast3(decT[kk][:, :], 16),
                    in1=repeat3(encbT[kk][:, c * 16:(c + 1) * 16], 64),
                )
                ht = hTp.tile([128, 1024], bf16, tag=f"hT{kk}",
                              name=f"hT{c}_{kk}", bufs=3)
                nc.scalar.activation(out=ht, in_=pre, func=Tanh)
                hts.append(ht)

            for kk in range(KK):
                pe_ = psum.tile([128, 512], f32, tag="g1", bufs=2,
                                name=f"g1e{kk}")
                for k in range(4):
                    nc.tensor.matmul(
                        pe_[:, :T],
                        lhsT=w1e_all[:, kk * 512 + k * 128:
                                     kk * 512 + (k + 1) * 128],
                        rhs=henc_all[:, k * T:(k + 1) * T],
                        start=(k == 0), stop=(k == 3),
                    )
                e_ = consts.tile([128, T], bf16, tag=f"encbT{kk}",
                                 name=f"encbT{kk}")
                nc.scalar.add(out=e_, in_=pe_[:, :T], add=b1_all[:, kk:kk + 1])
                encbT.append(e_)
                pd_ = psum.tile([128, 512], f32, tag="g1", bufs=2,
                                name=f"g1d{kk}")
                for k in range(4):
                    nc.tensor.matmul(
                        pd_[:, :U],
                        lhsT=w1d_all[:, kk * 512 + k * 128:
                                     kk * 512 + (k + 1) * 128],
                        rhs=hdec_all[:, k * U:(k + 1) * U],
                        start=(k == 0), stop=(k == 3),
                    )
                d_ = consts.tile([128, U], bf16, tag=f"decT{kk}",
                                 name=f"decT{kk}")
                nc.scalar.copy(out=d_, in_=pd_[:, :U])
                decT.append(d_)
                # chunk-0/1 builds for this kk follow its evacs immediately
                build_one(0, kk, hts0)
                build_one(1, kk, hts1)
                # keep the PE warm while the next kk's weights arrive
                warm_mms(2, f"g1_{kk}")

            hT_by_chunk = {0: hts0, 1: hts1}

            def emit_build(c):
                hts = []
                for kk in range(KK):
                    build_one(c, kk, hts)
                hT_by_chunk[c] = hts

            # bridge the GEMM1 -> GEMM2 handoff (builds still in flight)
            warm_mms(6, "bridge")

            for c in range(N_CHUNKS):
                if c + 2 < N_CHUNKS:
                    emit_build(c + 2)
                hts = hT_by_chunk.pop(c)
                last_c = c == N_CHUNKS - 1
                for pair in range(4):
                    ot = outp.tile([128, 2 * V], bf16, tag="out",
                                   name=f"out{c}_{pair}")
                    for half in range(2):
                        mt = pair * 2 + half
                        # final chunk, odd tiles: ScalarE pre-writes b2 into
                        # PSUM; matmuls accumulate onto it (has_written is
                        # still set from this bank's previous group).
                        act_evac = last_c and half == 1
                        ps2 = psum.tile([128, 1024], f32, tag="g2", bufs=3,
                                        name=f"ps2_{c}_{mt}")
                        if act_evac:
                            nc.scalar.copy(out=ps2, in_=b2_t)
                        for vc in range(2):
                            for kk in range(KK):
                                nc.tensor.matmul(
                                    ps2[:, vc * 512:(vc + 1) * 512],
                                    lhsT=hts[kk][:, mt * 128:(mt + 1) * 128],
                                    rhs=w2_all[:, kk * V + vc * 512:
                                               kk * V + (vc + 1) * 512],
                                    start=(kk == 0 and not act_evac),
                                    stop=(kk == KK - 1),
                                )
                        if act_evac:
                            nc.scalar.copy(out=ot[:, half * V:(half + 1) * V],
                                           in_=ps2)
                        else:
                            nc.vector.tensor_add(
                                out=ot[:, half * V:(half + 1) * V],
                                in0=ps2, in1=b2_t)
                    r0 = c * 1024 + pair * 256
                    ot_ap = ot[:, :]
                    in3 = bass.AP(tensor=ot_ap.tensor, offset=ot_ap.offset,
                                  ap=[ot_ap.ap[0], [V, 2], [1, V]])
                    o3 = bass.AP(tensor=out.tensor, offset=r0 * V,
                                 ap=[[V, 128], [128 * V, 2], [1, V]])
                    q = nc.sync if pair % 2 == 0 else nc.gpsimd
                    q.dma_start(out=o3, in_=in3)

    nc.finalize()
    return nc


def _get_nc():
    if "nc" not in _CACHE:
        _CACHE["nc"] = _build_bass()
    return _CACHE["nc"]


def _pack_w1_half(w1_half):
    """[HID, H] -> [128, KK*4*128] where chunk kk holds the 4 k-tiles of
    lhsT (partitions = the 2H contraction dim)."""
    arr = w1_half.reshape(KK, 128, 4, 128)  # [kk, q(out), k, p(contract)]
    return np.ascontiguousarray(
        arr.transpose(3, 0, 2, 1).reshape(128, KK * 4 * 128))


def _make_in_maps(h_enc, h_dec, W1, b1, W2, b2):
    h_enc = np.asarray(h_enc, dtype=np.float32)
    h_dec = np.asarray(h_dec, dtype=np.float32)
    W1 = np.asarray(W1, dtype=np.float32)
    b1 = np.asarray(b1, dtype=np.float32)
    W2 = np.asarray(W2, dtype=np.float32)
    b2 = np.asarray(b2, dtype=np.float32)

    w1eP = _pack_w1_half(W1[:, :H]).astype(BF16)
    w1dP = _pack_w1_half(W1[:, H:]).astype(BF16)
    w2T = np.ascontiguousarray(W2.T)
    w2P = np.concatenate([w2T[kk * 128:(kk + 1) * 128, :] for kk in range(KK)],
                         axis=1).astype(BF16)
    b1P = np.ascontiguousarray(b1.reshape(KK, 128).T)  # [128, KK] f32
    b2P = np.ascontiguousarray(
        np.tile(b2.reshape(1, V), (128, 1))).astype(BF16)

    in_maps = []
    for b in range(N_CORES):
        hencT = np.ascontiguousarray(h_enc[b].T)  # [H, T]
        hencP = hencT.reshape(4, 128, T).transpose(1, 0, 2).reshape(
            128, 4 * T).astype(BF16)
        hdecT = np.ascontiguousarray(h_dec[b].T)  # [H, U]
        hdecP = hdecT.reshape(4, 128, U).transpose(1, 0, 2).reshape(
            128, 4 * U).astype(BF16)
        in_maps.append({
            "hencP": np.ascontiguousarray(hencP),
            "hdecP": np.ascontiguousarray(hdecP),
            "w1eP": w1eP,
            "w1dP": w1dP,
            "w2P": w2P,
            "b1P": b1P,
            "b2P": b2P,
        })
    return in_maps


def _run(in_maps, **kwargs):
    from concourse import bass_utils
    nc = _get_nc()
    return bass_utils.run_bass_kernel_spmd(
        nc, in_maps, core_ids=list(range(N_CORES)), **kwargs)


def kernel(h_enc, h_dec, W1, b1, W2, b2):
    in_maps = _make_in_maps(h_enc, h_dec, W1, b1, W2, b2)
    res = _run(in_maps)
    outs = [r["out"].reshape(T, U, V).astype(np.float32)
            for r in res.results]
    return np.stack(outs, axis=0)


# revision 7
# speedup vs baseline: 1.0237x; 1.0073x over previous
"""Trainium2 Bass kernel for the RNN-T style Joint network:

    out[b,t,u,v] = sum_k tanh(enc_p[b,t,k] + dec_p[b,u,k] + b1[k]) * W2[v,k] + b2[v]
    enc_p = h_enc @ W1[:, :H].T ; dec_p = h_dec @ W1[:, H:].T

Sharding: data-parallel over B across 8 NeuronCores (B == 8, one batch row per
core). Weights are replicated. No collectives needed.

Per-core pipeline (one NeuronCore):
  warmup (PE): dummy N=512 matmuls during the input-DMA wait keep the HAM
      clock gate at K=8/8 from the start.
  input DMA: W1 packed host-side by GEMM1 k-tile (kk); chunks balanced over
      the sync/scalar/gpsimd queues (each queue's transfers serialize at
      ~85GB/s and only start ~8us in, so the critical path is per-queue
      bytes, not issue count).
  GEMM1 (PE): per kk as its weights land: enc_pT [HID, T] and dec_pT [HID, U]
      on 2 rotating PSUM banks, b1 folded via ScalarE bias during bf16 evac.
      Build adds for chunks 0/1 interleave per-kk so GEMM2 starts early.
  broadcast-add (VectorE, bf16): pre[j, t'*64+u] = encbT[j, t] + decT[j, u],
      one tensor_add per [128, 1024] chunk via stride-0 broadcast APs.
  tanh (ScalarE): bf16 -> bf16 hT tiles (stationary operand of GEMM2).
  GEMM2 (PE, bf16): 1280 N=512 matmuls (~216ns each, the roofline term) with
      5-K-tile accumulation into fp32 PSUM (bf16 PSUM is TRN3-only).
  b2 + evac (VectorE): PSUM + b2 -> bf16 out tile [128, 2048].
  DMA out: one 512KB store per two mt blocks, round-robin over three queues;
      the final pair splits into two parallel single-block stores.

Output is written bf16 and upcast to fp32 on the host (adds ~0.1% rms noise;
well inside the 2e-2 gate) to halve the HBM write traffic and SBUF footprint.
"""

import numpy as np
import ml_dtypes

B, T, U, H = 8, 256, 64, 512
HID, V = 640, 1024
TU = T * U  # 16384
N_CORES = 8
N_CHUNKS = TU // 1024  # 16 chunks of 16 t-values x 64 u-values
KK = HID // 128  # 5 K-tiles

BF16 = ml_dtypes.bfloat16

_CACHE = {}


def _build_bass():
    import concourse.bass as bass
    import concourse.tile as tile
    from concourse import bacc, mybir

    f32 = mybir.dt.float32
    bf16 = mybir.dt.bfloat16
    Tanh = mybir.ActivationFunctionType.Tanh

    nc = bacc.Bacc("TRN2", target_bir_lowering=False, debug=False,
                   num_devices=N_CORES)

    # W1 enc/dec halves packed as [128, KK, 4, 128] -> [128, KK*512]
    w1eP = nc.dram_tensor("w1eP", [128, KK * 512], bf16, kind="ExternalInput").ap()
    w1dP = nc.dram_tensor("w1dP", [128, KK * 512], bf16, kind="ExternalInput").ap()
    hencP = nc.dram_tensor("hencP", [128, 4 * T], bf16, kind="ExternalInput").ap()
    hdecP = nc.dram_tensor("hdecP", [128, 4 * U], bf16, kind="ExternalInput").ap()
    w2P = nc.dram_tensor("w2P", [128, KK * V], bf16, kind="ExternalInput").ap()
    b1P = nc.dram_tensor("b1P", [128, KK], f32, kind="ExternalInput").ap()
    b2P = nc.dram_tensor("b2P", [128, V], bf16, kind="ExternalInput").ap()
    out = nc.dram_tensor("out", [TU, V], bf16, kind="ExternalOutput").ap()

    def bcast3(ap2d, mid):
        """[P, N] AP -> [P, mid, N] with a stride-0 middle dim."""
        return bass.AP(tensor=ap2d.tensor, offset=ap2d.offset,
                       ap=[ap2d.ap[0], [0, mid], ap2d.ap[1]])

    def repeat3(ap2d, inner):
        """[P, N] AP -> [P, N, inner] with a stride-0 inner dim."""
        return bass.AP(tensor=ap2d.tensor, offset=ap2d.offset,
                       ap=[ap2d.ap[0], ap2d.ap[1], [0, inner]])

    with tile.TileContext(nc) as tc:
        with (
            tc.tile_pool(name="consts", bufs=1) as consts,
            tc.tile_pool(name="psum", bufs=1, space="PSUM") as psum,
            tc.tile_pool(name="prep", bufs=2) as prep,
            tc.tile_pool(name="hTp", bufs=4) as hTp,
            tc.tile_pool(name="outp", bufs=4) as outp,
        ):
            # ---- scratch + PE warmup during the input-DMA wait ----
            wk = consts.tile([128, 512], bf16, tag="wk", name="wk")
            nc.vector.memset(wk, 0)

            def warm_mms(n, label):
                for i in range(n):
                    pw = psum.tile([128, 512], f32, tag="g1", bufs=2,
                                   name=f"warm_{label}_{i}")
                    nc.tensor.matmul(pw, lhsT=wk[:, :128], rhs=wk,
                                     start=True, stop=True)

            warm_mms(12, "pre")

            # ---- input DMAs: balanced over the three DMA-capable queues ----
            henc_all = consts.tile([128, 4 * T], bf16, tag="henc", name="henc")
            hdec_all = consts.tile([128, 4 * U], bf16, tag="hdec", name="hdec")
            b1_all = consts.tile([128, KK], f32, tag="b1", name="b1")
            w1e_all = consts.tile([128, KK * 512], bf16, tag="w1e", name="w1e")
            w1d_all = consts.tile([128, KK * 512], bf16, tag="w1d", name="w1d")
            w2_all = consts.tile([128, KK * V], bf16, tag="w2", name="w2")
            b2_t = consts.tile([128, V], bf16, tag="b2", name="b2")

            def w1e_dma(q, kk):
                q.dma_start(out=w1e_all[:, kk * 512:(kk + 1) * 512],
                            in_=w1eP[:, kk * 512:(kk + 1) * 512])

            def w1d_dma(q, kk):
                q.dma_start(out=w1d_all[:, kk * 512:(kk + 1) * 512],
                            in_=w1dP[:, kk * 512:(kk + 1) * 512])

            def w2_dma(q, kk):
                q.dma_start(out=w2_all[:, kk * V:(kk + 1) * V],
                            in_=w2P[:, kk * V:(kk + 1) * V])

            # sync queue
            nc.sync.dma_start(out=henc_all[:, 2 * T:], in_=hencP[:, 2 * T:])
            w1e_dma(nc.sync, 0)
            w1e_dma(nc.sync, 2)
            w1d_dma(nc.sync, 3)
            w2_dma(nc.sync, 3)
            w2_dma(nc.sync, 4)
            # scalar queue
            nc.scalar.dma_start(out=henc_all[:, :2 * T], in_=hencP[:, :2 * T])
            w1e_dma(nc.scalar, 1)
            w1e_dma(nc.scalar, 3)
            w1d_dma(nc.scalar, 4)
            w2_dma(nc.scalar, 1)
            w2_dma(nc.scalar, 2)
            nc.scalar.dma_start(out=b2_t, in_=b2P[:, :])
            # gpsimd queue
            nc.gpsimd.dma_start(out=b1_all, in_=b1P[:, :])
            nc.gpsimd.dma_start(out=hdec_all, in_=hdecP[:, :])
            w1d_dma(nc.gpsimd, 0)
            w1d_dma(nc.gpsimd, 1)
            w1d_dma(nc.gpsimd, 2)
            w1e_dma(nc.gpsimd, 4)
            w2_dma(nc.gpsimd, 0)

            # ---- GEMM1 per kk (as weights land) + early builds ----
            encbT = []
            decT = []
            hts0 = []
            hts1 = []

            def build_one(c, kk, hts):
                pre = prep.tile([128, 1024], bf16, tag=f"pre{kk}",
                                name=f"pre{c}_{kk}", bufs=2)
                pre_ap = pre[:, :]
                out3 = bass.AP(tensor=pre_ap.tensor, offset=pre_ap.offset,
                               ap=[pre_ap.ap[0], [64, 16], [1, 64]])
                nc.vector.tensor_add(
                    out=out3,
                    in0=bcast3(decT[kk][:, :], 16),
                    in1=repeat3(encbT[kk][:, c * 16:(c + 1) * 16], 64),
                )
                ht = hTp.tile([128, 1024], bf16, tag=f"hT{kk}",
                              name=f"hT{c}_{kk}", bufs=4)
                nc.scalar.activation(out=ht, in_=pre, func=Tanh)
                hts.append(ht)

            for kk in range(KK):
                pe_ = psum.tile([128, 512], f32, tag="g1", bufs=2,
                                name=f"g1e{kk}")
                for k in range(4):
                    nc.tensor.matmul(
                        pe_[:, :T],
                        lhsT=w1e_all[:, kk * 512 + k * 128:
                                     kk * 512 + (k + 1) * 128],
                        rhs=henc_all[:, k * T:(k + 1) * T],
                        start=(k == 0), stop=(k == 3),
                    )
                e_ = consts.tile([128, T], bf16, tag=f"encbT{kk}",
                                 name=f"encbT{kk}")
                nc.scalar.add(out=e_, in_=pe_[:, :T], add=b1_all[:, kk:kk + 1])
                encbT.append(e_)
                pd_ = psum.tile([128, 512], f32, tag="g1", bufs=2,
                                name=f"g1d{kk}")
                for k in range(4):
                    nc.tensor.matmul(
                        pd_[:, :U],
                        lhsT=w1d_all[:, kk * 512 + k * 128:
                                     kk * 512 + (k + 1) * 128],
                        rhs=hdec_all[:, k * U:(k + 1) * U],
                        start=(k == 0), stop=(k == 3),
                    )
                d_ = consts.tile([128, U], bf16, tag=f"decT{kk}",
                                 name=f"decT{kk}")
                nc.scalar.copy(out=d_, in_=pd_[:, :U])
                decT.append(d_)
                # chunk-0/1 builds for this kk follow its evacs immediately
                build_one(0, kk, hts0)
                build_one(1, kk, hts1)
                # keep the PE warm while the next kk's weights arrive
                warm_mms(2, f"g1_{kk}")

            hT_by_chunk = {0: hts0, 1: hts1}

            def emit_build(c):
                hts = []
                for kk in range(KK):
                    build_one(c, kk, hts)
                hT_by_chunk[c] = hts

            # bridge the GEMM1 -> GEMM2 handoff (builds still in flight)
            warm_mms(6, "bridge")
            emit_build(2)

            for c in range(N_CHUNKS):
                hts = hT_by_chunk.pop(c)
                for pair in range(4):
                    if pair == 2 and c + 3 < N_CHUNKS:
                        emit_build(c + 3)
                    ot = outp.tile([128, 2 * V], bf16, tag="out",
                                   name=f"out{c}_{pair}")
                    for half in range(2):
                        mt = pair * 2 + half
                        ps2 = psum.tile([128, 1024], f32, tag="g2", bufs=3,
                                        name=f"ps2_{c}_{mt}")
                        for vc in range(2):
                            for kk in range(KK):
                                nc.tensor.matmul(
                                    ps2[:, vc * 512:(vc + 1) * 512],
                                    lhsT=hts[kk][:, mt * 128:(mt + 1) * 128],
                                    rhs=w2_all[:, kk * V + vc * 512:
                                               kk * V + (vc + 1) * 512],
                                    start=(kk == 0), stop=(kk == KK - 1),
                                )
                        nc.vector.tensor_add(
                            out=ot[:, half * V:(half + 1) * V],
                            in0=ps2, in1=b2_t)
                    r0 = c * 1024 + pair * 256
                    ot_ap = ot[:, :]
                    if c == N_CHUNKS - 1 and pair == 3:
                        # split the final store across two queues
                        for half in range(2):
                            i2 = bass.AP(tensor=ot_ap.tensor,
                                         offset=ot_ap.offset + half * V,
                                         ap=[ot_ap.ap[0], [1, V]])
                            o2 = out[r0 + half * 128:r0 + (half + 1) * 128, :]
                            (nc.sync if half == 0 else nc.gpsimd).dma_start(
                                out=o2, in_=i2)
                    else:
                        in3 = bass.AP(tensor=ot_ap.tensor, offset=ot_ap.offset,
                                      ap=[ot_ap.ap[0], [V, 2], [1, V]])
                        o3 = bass.AP(tensor=out.tensor, offset=r0 * V,
                                     ap=[[V, 128], [128 * V, 2], [1, V]])
                        q = (nc.sync, nc.gpsimd, nc.scalar)[(c * 4 + pair) % 3]
                        q.dma_start(out=o3, in_=in3)

    nc.finalize()
    return nc


def _get_nc():
    if "nc" not in _CACHE:
        _CACHE["nc"] = _build_bass()
    return _CACHE["nc"]


def _pack_w1_half(w1_half):
    """[HID, H] -> [128, KK*4*128] where chunk kk holds the 4 k-tiles of
    lhsT (partitions = the 2H contraction dim)."""
    arr = w1_half.reshape(KK, 128, 4, 128)  # [kk, q(out), k, p(contract)]
    return np.ascontiguousarray(
        arr.transpose(3, 0, 2, 1).reshape(128, KK * 4 * 128))


def _make_in_maps(h_enc, h_dec, W1, b1, W2, b2):
    h_enc = np.asarray(h_enc, dtype=np.float32)
    h_dec = np.asarray(h_dec, dtype=np.float32)
    W1 = np.asarray(W1, dtype=np.float32)
    b1 = np.asarray(b1, dtype=np.float32)
    W2 = np.asarray(W2, dtype=np.float32)
    b2 = np.asarray(b2, dtype=np.float32)

    w1eP = _pack_w1_half(W1[:, :H]).astype(BF16)
    w1dP = _pack_w1_half(W1[:, H:]).astype(BF16)
    w2T = np.ascontiguousarray(W2.T)
    w2P = np.concatenate([w2T[kk * 128:(kk + 1) * 128, :] for kk in range(KK)],
                         axis=1).astype(BF16)
    b1P = np.ascontiguousarray(b1.reshape(KK, 128).T)  # [128, KK] f32
    b2P = np.ascontiguousarray(
        np.tile(b2.reshape(1, V), (128, 1))).astype(BF16)

    in_maps = []
    for b in range(N_CORES):
        hencT = np.ascontiguousarray(h_enc[b].T)  # [H, T]
        hencP = hencT.reshape(4, 128, T).transpose(1, 0, 2).reshape(
            128, 4 * T).astype(BF16)
        hdecT = np.ascontiguousarray(h_dec[b].T)  # [H, U]
        hdecP = hdecT.reshape(4, 128, U).transpose(1, 0, 2).reshape(
            128, 4 * U).astype(BF16)
        in_maps.append({
            "hencP": np.ascontiguousarray(hencP),
            "hdecP": np.ascontiguousarray(hdecP),
            "w1eP": w1eP,
            "w1dP": w1dP,
            "w2P": w2P,
            "b1P": b1P,
            "b2P": b2P,
        })
    return in_maps


def _run(in_maps, **kwargs):
    from concourse import bass_utils
    nc = _get_nc()
    return bass_utils.run_bass_kernel_spmd(
        nc, in_maps, core_ids=list(range(N_CORES)), **kwargs)


def kernel(h_enc, h_dec, W1, b1, W2, b2):
    in_maps = _make_in_maps(h_enc, h_dec, W1, b1, W2, b2)
    res = _run(in_maps)
    outs = [r["out"].reshape(T, U, V).astype(np.float32)
            for r in res.results]
    return np.stack(outs, axis=0)
